# revision 2
# baseline (speedup 1.0000x reference)
"""Trainium2 Bass kernel v3 for nn_BatchedHomoModel_22179211116720 (GNN message passing).

Per core (data-parallel over seeds, SPMD single program):
  - L1 + L2A(emb-side of conv2) edge gathers via banked dma_gather (int16 idx,
    32768-row banks of the 500K emb table), dense layout [edge%128, edge//128].
  - Routing to per-dst-tile PSUM via one-hot matrices built with is_equal
    against host-provided slot planes; mid aggregation in 2 PSUM passes.
  - Mid epilogue: @W0, r1-scale + Lrelu -> hsum table (bf16) in DRAM.
  - L2B: single dma_gather from hsum (slot layout, K2 schedule), diag(s2)
    matmuls into the same seed PSUM as L2A; seed epilogue @W1 + r2 + h0.
"""

import hashlib
from contextlib import ExitStack

import ml_dtypes
import numpy as np

import concourse.tile as tile
from concourse import bacc, bass, mybir
from concourse.bass_utils import run_bass_kernel_spmd

P = 128
D = 128
NCORES = 8
BANK = 32768
SENT = 300.0  # plane sentinel (never equals iota 0..127)
ALPHA = 0.01

F32 = mybir.dt.float32
BF16 = mybir.dt.bfloat16
I32 = mybir.dt.int32
I16 = mybir.dt.int16
NP_BF16 = ml_dtypes.bfloat16


def _rsqrt_deg(counts):
    return (1.0 / np.sqrt(np.maximum(counts, 1).astype(np.float64))).astype(np.float32)


def _group_positions(keys, num_keys):
    order = np.argsort(keys, kind="stable")
    sorted_keys = keys[order]
    counts = np.bincount(keys, minlength=num_keys)
    starts = np.concatenate([[0], np.cumsum(counts)[:-1]])
    pos_sorted = np.arange(keys.shape[0]) - starts[sorted_keys]
    pos = np.empty_like(pos_sorted)
    pos[order] = pos_sorted
    return pos


def build_plan(inputs, ncores=NCORES, tiles_per_pass=24):
    emb_rows = int(np.asarray(inputs["emb"]).shape[0])
    nbanks = (emb_rows + BANK - 1) // BANK
    nid_src1 = np.asarray(inputs["nid_src1"]).astype(np.int64)
    nid_src2 = np.asarray(inputs["nid_src2"]).astype(np.int64)
    nid_dst2 = np.asarray(inputs["nid_dst2"]).astype(np.int64)
    e1_src = np.asarray(inputs["e1_src"]).astype(np.int64)
    e1_dst = np.asarray(inputs["e1_dst"]).astype(np.int64)
    e2_src = np.asarray(inputs["e2_src"]).astype(np.int64)
    e2_dst = np.asarray(inputs["e2_dst"]).astype(np.int64)

    N1, N2, B = nid_src1.shape[0], nid_src2.shape[0], nid_dst2.shape[0]
    assert B % (P * ncores) == 0
    T2 = B // (P * ncores)

    cnt_out1 = np.bincount(e1_src, minlength=N1)
    cnt_in1 = np.bincount(e1_dst, minlength=N2)
    cnt_out2 = np.bincount(e2_src, minlength=N2)
    cnt_in2 = np.bincount(e2_dst, minlength=B)
    s1_edge = _rsqrt_deg(cnt_out1)[e1_src]
    s2_edge = _rsqrt_deg(cnt_out2)[e2_src]
    r1_node = _rsqrt_deg(cnt_in1)
    r2_node = _rsqrt_deg(cnt_in2)

    # seed dealing (degree-major for the K2 slot schedule of L2B)
    seed_perm = np.argsort(-cnt_in2, kind="stable")
    rank_of_seed = np.empty(B, np.int64)
    rank_of_seed[seed_perm] = np.arange(B)
    K2 = []
    for lt in range(T2):
        K2.append(max(1, int(cnt_in2[seed_perm[P * ncores * lt]])))
    off2 = np.concatenate([[0], np.cumsum(K2)]).astype(np.int64)
    n2sub = int(off2[-1])

    r_e = rank_of_seed[e2_dst]
    k_e = _group_positions(r_e, B)
    p_e = r_e % P
    g_e = r_e // P
    core_e = g_e % ncores
    lt_e = g_e // ncores
    su_e = off2[lt_e] + k_e

    # per-core mids
    mids_per_core = []
    for c in range(ncores):
        mids = np.unique(e2_src[core_e == c])
        mids_per_core.append(mids)
    T1 = max((len(m) + P - 1) // P for m in mids_per_core)
    Mpad = T1 * P
    assert Mpad <= 32768, "hsum table must fit one int16 bank"
    n_pass0 = min(tiles_per_pass, T1)
    NRT = T1 + T2  # routing tiles: L1 mid tiles then L2A seed tiles
    pass_of_rtile = np.array(
        [0 if t < n_pass0 else 1 for t in range(T1)] + [0] * T2, np.int64
    )
    npasses = 2 if T1 > n_pass0 else 1

    # ---- per-core dense edge sets: (pass, bank, rtile) -> edges ----
    per_core = []
    for c in range(ncores):
        mids = mids_per_core[c]
        n_mid = len(mids)
        mid2local = np.full(N2, -1, np.int64)
        mid2local[mids] = np.arange(n_mid)

        lm = mid2local[e1_dst]
        sel = np.nonzero(lm >= 0)[0]
        l1_row = nid_src1[e1_src[sel]]
        l1_rtile = lm[sel] // P
        l1_slot = lm[sel] % P
        l1_s = s1_edge[sel]

        sel2 = np.nonzero(core_e == c)[0]
        l2_row = nid_src2[e2_src[sel2]]
        l2_rtile = T1 + lt_e[sel2]
        l2_slot = p_e[sel2]
        l2_s = s2_edge[sel2]

        row = np.concatenate([l1_row, l2_row])
        rtile = np.concatenate([l1_rtile, l2_rtile])
        slot = np.concatenate([l1_slot, l2_slot])
        s = np.concatenate([l1_s, l2_s])
        bank = row // BANK
        pss = pass_of_rtile[rtile]
        key = (pss * nbanks + bank) * NRT + rtile
        order = np.argsort(key, kind="stable")
        per_core.append(
            dict(
                row=row[order], rtile=rtile[order], slot=slot[order],
                s=s[order], key=key[order],
                mid2local=mid2local, n_mid=n_mid, mids=mids,
            )
        )

    # shared capacity schedule: N[pass, bank, rtile] = max over cores
    nkeys = npasses * nbanks * NRT
    Ncap = np.zeros(nkeys, np.int64)
    for c in range(ncores):
        cnt = np.bincount(per_core[c]["key"], minlength=nkeys)
        Ncap = np.maximum(Ncap, cnt)
    # every rtile needs >= 1 edge slot somewhere (PSUM region init)
    for r in range(NRT):
        p = pass_of_rtile[r]
        if Ncap[(p * nbanks + 0) * NRT + r : (p * nbanks + nbanks) * NRT : NRT].sum() == 0:
            Ncap[(p * nbanks + 0) * NRT + r] = 1

    # ---- build shared call/column schedule ----
    calls = []  # dict(pass, bank, cols, col_rtiles, nplanes, c0, i0)
    tot_cols = 0
    mm_seq = {}  # rtile -> list of (call_idx, col_in_call, plane)
    for pss in range(npasses):
        for b in range(nbanks):
            runs = []
            for r in range(NRT):
                if pass_of_rtile[r] != pss:
                    continue
                n = int(Ncap[(pss * nbanks + b) * NRT + r])
                if n:
                    runs.append((r, n))
            total = sum(n for _, n in runs)
            if total == 0:
                continue
            cols = (total + P - 1) // P
            # column -> pieces [(rtile, pos_in_flat, cnt)]
            col_pieces = [[] for _ in range(cols)]
            pos = 0
            for r, n in runs:
                while n > 0:
                    col = pos // P
                    take = min(n, (col + 1) * P - pos)
                    col_pieces[col].append((r, pos, take))
                    pos += take
                    n -= take
            col_rtiles = []
            nplanes = 0
            ci = len(calls)
            for col in range(cols):
                ents = []
                for k, (r, _, _) in enumerate(col_pieces[col]):
                    ents.append((k, r))
                    mm_seq.setdefault(r, []).append((ci, col, k))
                col_rtiles.append(ents)
                nplanes = max(nplanes, len(ents))
            # per-plane column range (build R_k only where a k-th piece exists)
            k_rng = []
            for k in range(nplanes):
                cls = [col for col in range(cols) if len(col_rtiles[col]) > k]
                k_rng.append((min(cls), max(cls) + 1))
            calls.append(
                dict(
                    pss=pss, bank=b, cols=cols, col_rtiles=col_rtiles,
                    nplanes=nplanes, c0=tot_cols, runs=runs,
                    col_pieces=col_pieces, k_rng=k_rng,
                )
            )
            tot_cols += cols
    NPL = max(c["nplanes"] for c in calls)
    CMAX = max(c["cols"] for c in calls)

    # start/stop flags are PER PSUM BANK: matmul start=True lazily zeroes the
    # whole 2KB bank (zero region), so emit exactly one start (first MM into
    # the bank each pass) and one stop (last MM). The seed bank (agg2) gets
    # its stop later in L2B.
    def bank_key(r):
        if r >= T1:
            return ("agg2",)
        p = pass_of_rtile[r]
        lt = r - (0 if p == 0 else n_pass0)
        return (p, lt // 4)

    bank_seq = {}
    for r, lst in mm_seq.items():
        for ent in lst:
            bank_seq.setdefault(bank_key(r), []).append(ent)
    # order by emission: calls are iterated in order, cols in order, planes in order
    mm_flags = {}
    for bk, lst in bank_seq.items():
        lst_sorted = sorted(lst)
        for j, ent in enumerate(lst_sorted):
            st = j == 0
            sp = (j == len(lst_sorted) - 1) and bk != ("agg2",)
            mm_flags[ent] = (st, sp)

    # ---- per-core dense tensors ----
    for c in range(ncores):
        pc = per_core[c]
        cnt = np.bincount(pc["key"], minlength=nkeys)
        starts = np.concatenate([[0], np.cumsum(cnt)])
        rowloc = np.zeros((P, tot_cols), np.int32)
        planes = np.full((NPL, P, tot_cols), SENT, np.float32)
        s_all = np.ones((P, tot_cols), np.float32)
        for call in calls:
            pss, b = call["pss"], call["bank"]
            for r, pos, take in (pce for pieces in call["col_pieces"] for pce in pieces):
                pass  # placeholder (vectorized below)
        # vectorized fill: for each run of each call, compute flat positions
        for call in calls:
            pss, b = call["pss"], call["bank"]
            flat0 = call["c0"] * P
            pos = 0
            for r, ncap in call["runs"]:
                k = (pss * nbanks + b) * NRT + r
                n_c = int(cnt[k])
                src0 = int(starts[k])
                if n_c:
                    fp = flat0 + pos + np.arange(n_c)
                    pp, cc = fp % P, fp // P
                    rows = pc["row"][src0 : src0 + n_c]
                    rowloc[pp, cc] = (rows - b * BANK).astype(np.int32)
                    s_all[pp, cc] = pc["s"][src0 : src0 + n_c]
                    # plane index of rtile r in each column
                    slots = pc["slot"][src0 : src0 + n_c]
                    for col in np.unique(cc):
                        kk = next(
                            k2 for k2, (r2, _, _) in enumerate(
                                call["col_pieces"][col - call["c0"]]
                            ) if r2 == r
                        )
                        m = cc == col
                        planes[kk, pp[m], col] = slots[m]
                pos += ncap
        pc["rowloc"] = rowloc
        pc["planes"] = planes
        pc["s_all"] = s_all

        # L2B slot metadata
        s2meta = np.zeros((P, n2sub), np.float32)
        gB = np.zeros((P, n2sub), np.int32)
        sel2 = np.nonzero(core_e == c)[0]
        s2meta[p_e[sel2], su_e[sel2]] = s2_edge[sel2]
        gB[p_e[sel2], su_e[sel2]] = pc["mid2local"][e2_src[sel2]]
        pc["s2meta"] = s2meta
        pc["gidxB"] = gB

        r1row = np.ones(Mpad, np.float32)
        r1row[: pc["n_mid"]] = r1_node[pc["mids"]]
        pc["r1meta"] = r1row.reshape(T1, P).T.copy()

        lt_grid, p_grid = np.meshgrid(np.arange(T2), np.arange(P), indexing="ij")
        ranks = (c + ncores * lt_grid) * P + p_grid
        orig = seed_perm[ranks]
        pc["h0meta"] = nid_dst2[orig].T.astype(np.int32).copy()
        pc["r2meta"] = r2_node[orig].T.astype(np.float32).copy()
        pc["orig_seeds"] = orig

    # idx wrap per call (i = scol*16 + p over first 16 partitions, replicated)
    for c in range(ncores):
        pc = per_core[c]
        idx_all = np.zeros((P, tot_cols * 8), np.int16)
        for call in calls:
            c0, cols = call["c0"], call["cols"]
            flat = pc["rowloc"][:, c0 : c0 + cols].T.reshape(-1)  # i = col*128+p
            blk = flat.reshape(cols * 8, 16).T.astype(np.int16)  # [16, cols*8]
            for g in range(8):
                idx_all[g * 16 : (g + 1) * 16, c0 * 8 : (c0 + cols) * 8] = blk
        pc["idx_all"] = idx_all

        gBf = pc["gidxB"].T.reshape(-1)  # slot seq i = su*128 + p
        idxB = np.zeros((P, n2sub * 8), np.int16)
        blk = gBf.reshape(n2sub * 8, 16).T.astype(np.int16)
        for g in range(8):
            idxB[g * 16 : (g + 1) * 16] = blk
        pc["idxB"] = idxB

    return dict(
        cores=per_core, calls=calls, mm_flags=mm_flags, NPL=NPL, CMAX=CMAX,
        tot_cols=tot_cols, T1=T1, T2=T2, NRT=NRT, Mpad=Mpad, n_pass0=n_pass0,
        npasses=npasses, nbanks=nbanks, K2=[int(k) for k in K2],
        off2=[int(x) for x in off2], n2sub=n2sub, ncores=ncores, B=B,
        emb_rows=emb_rows, seed_perm=seed_perm,
    )


def leaky(x):
    return np.where(x >= 0, x, ALPHA * x).astype(np.float32)


def simulate_plan(inputs, plan):
    """Numpy mirror (f32) of the device program."""
    emb = np.asarray(inputs["emb"], np.float32)
    W0 = np.asarray(inputs["W0"], np.float32)
    b0 = np.asarray(inputs["b0"], np.float32)
    W1 = np.asarray(inputs["W1"], np.float32)
    b1 = np.asarray(inputs["b1"], np.float32)
    T1, T2, NRT = plan["T1"], plan["T2"], plan["NRT"]
    K2, off2 = plan["K2"], plan["off2"]
    out = np.zeros((plan["B"], D), np.float32)
    for c, pc in enumerate(plan["cores"]):
        agg = np.zeros((NRT, D, P), np.float32)  # [rtile, feat, slot]
        for call in plan["calls"]:
            b, c0, cols = call["bank"], call["c0"], call["cols"]
            X = emb[pc["rowloc"][:, c0 : c0 + cols].astype(np.int64) + b * BANK]
            Y = leaky(X) * pc["s_all"][:, c0 : c0 + cols, None]  # [p, col, D]
            for col in range(cols):
                for k, r in call["col_rtiles"][col]:
                    pl = pc["planes"][k, :, c0 + col]
                    R = (pl[:, None] == np.arange(P)[None, :]).astype(np.float32)
                    agg[r] += Y[:, col, :].T @ R
        # mid epilogues -> hsum
        hsum = np.zeros((plan["Mpad"], D), np.float32)
        for t in range(T1):
            z = (agg[t].T @ W0) * pc["r1meta"][:, t, None] + b0  # [slot, D]
            hsum[t * P : (t + 1) * P] = leaky(z)
        # L2B
        XB = hsum[pc["gidxB"]] * pc["s2meta"][:, :, None]  # [p, su, D]
        for u in range(T2):
            agg2 = agg[T1 + u].T + XB[:, off2[u] : off2[u + 1], :].sum(axis=1)
            z2 = (agg2 @ W1) * pc["r2meta"][:, u, None] + 2.0 * b1
            out[pc["orig_seeds"][u]] = z2 + emb[pc["h0meta"][:, u]]
    return out


def build_nc(plan, has_b0, has_b1, num_devices=None, debug_dump=False):
    T1, T2, NRT = plan["T1"], plan["T2"], plan["NRT"]
    NPL, CMAX = plan["NPL"], plan["CMAX"]
    tot_cols, Mpad = plan["tot_cols"], plan["Mpad"]
    n_pass0, npasses = plan["n_pass0"], plan["npasses"]
    K2, off2, n2sub = plan["K2"], plan["off2"], plan["n2sub"]
    emb_rows = plan["emb_rows"]
    if num_devices is None:
        num_devices = plan["ncores"]
    mm_flags = plan["mm_flags"]
    tile_of_su2 = np.repeat(np.arange(T2), K2)

    nc = bacc.Bacc("TRN2", target_bir_lowering=False, debug=False,
                   enable_asserts=False, num_devices=num_devices)

    emb_d = nc.dram_tensor("emb", (emb_rows, D), F32, kind="ExternalInput").ap()
    W0_d = nc.dram_tensor("W0bf", (D, D), BF16, kind="ExternalInput").ap()
    W1_d = nc.dram_tensor("W1bf", (D, D), BF16, kind="ExternalInput").ap()
    ident_d = nc.dram_tensor("identbf", (P, P), BF16, kind="ExternalInput").ap()
    iota_d = nc.dram_tensor("iotabf", (P, P), BF16, kind="ExternalInput").ap()
    idx_d = nc.dram_tensor("idx_all", (P, tot_cols * 8), I16, kind="ExternalInput").ap()
    pl_d = [
        nc.dram_tensor(f"plane{k}", (P, tot_cols), BF16, kind="ExternalInput").ap()
        for k in range(NPL)
    ]
    s_d = nc.dram_tensor("s_all", (P, tot_cols), BF16, kind="ExternalInput").ap()
    idxB_d = nc.dram_tensor("idxB", (P, n2sub * 8), I16, kind="ExternalInput").ap()
    s2_d = nc.dram_tensor("s2bf", (P, n2sub), BF16, kind="ExternalInput").ap()
    r1_d = nc.dram_tensor("r1meta", (P, T1), F32, kind="ExternalInput").ap()
    h0_d = nc.dram_tensor("h0meta", (P, T2), I32, kind="ExternalInput").ap()
    r2_d = nc.dram_tensor("r2meta", (P, T2), F32, kind="ExternalInput").ap()
    if has_b0:
        b0bc_d = nc.dram_tensor("b0bc", (P, D), F32, kind="ExternalInput").ap()
    if has_b1:
        b1bc_d = nc.dram_tensor("b1bc", (P, D), F32, kind="ExternalInput").ap()
    out_d = nc.dram_tensor("out", (T2 * P, D), F32, kind="ExternalOutput").ap()
    hsum_d = nc.dram_tensor("hsum", (Mpad, D), BF16, kind="Internal").ap()
    if debug_dump:
        aggsb_d = nc.dram_tensor("aggsb_dbg", (P, T1 * P), F32,
                                 kind="ExternalOutput").ap()
        hsum_dbg_d = nc.dram_tensor("hsum_dbg", (Mpad, D), F32,
                                    kind="ExternalOutput").ap()
        xB_dbg_d = nc.dram_tensor("xB_dbg", (P, n2sub * P), F32,
                                  kind="ExternalOutput").ap()
        agg2_dbg_d = nc.dram_tensor("agg2_dbg", (P, T2 * P), F32,
                                    kind="ExternalOutput").ap()
        xg0_d = nc.dram_tensor("xg0_dbg", (P, CMAX * P), F32,
                               kind="ExternalOutput").ap()
        y0_d = nc.dram_tensor("y0_dbg", (P, CMAX * P), F32,
                              kind="ExternalOutput").ap()
        R00_d = nc.dram_tensor("R00_dbg", (P, CMAX * P), F32,
                               kind="ExternalOutput").ap()

    AX = bass.IndirectOffsetOnAxis
    LR = mybir.ActivationFunctionType.Lrelu
    CP = mybir.ActivationFunctionType.Copy

    with tile.TileContext(nc) as tc, ExitStack() as ctx:
        cpool = ctx.enter_context(tc.tile_pool(name="const", bufs=1))
        wpool = ctx.enter_context(tc.tile_pool(name="work", bufs=3))
        spool = ctx.enter_context(tc.tile_pool(name="small", bufs=3))
        ppool = ctx.enter_context(tc.tile_pool(name="psum", bufs=1, space="PSUM"))

        def load_const(ap_d, dtype):
            nm = "c_" + ap_d.name
            t = cpool.tile(list(ap_d.shape), dtype, name=nm, tag=nm)
            nc.sync.dma_start(out=t[:], in_=ap_d[:])
            return t

        W0_s = load_const(W0_d, BF16)
        W1_s = load_const(W1_d, BF16)
        ident = load_const(ident_d, BF16)
        iota = load_const(iota_d, BF16)
        idx_all = load_const(idx_d, I16)
        planes = [load_const(pl_d[k], BF16) for k in range(NPL)]
        s_all = load_const(s_d, BF16)
        idxB = load_const(idxB_d, I16)
        s2 = load_const(s2_d, BF16)
        r1m = load_const(r1_d, F32)
        h0m = load_const(h0_d, I32)
        r2m = load_const(r2_d, F32)
        b0bc = load_const(b0bc_d, F32) if has_b0 else None
        b1bc = load_const(b1bc_d, F32) if has_b1 else None

        aggsb = cpool.tile([P, T1 * P], BF16, name="aggsb", tag="aggsb")
        agg2sb = cpool.tile([P, T2 * P], BF16, name="agg2sb", tag="agg2sb")
        h0buf = cpool.tile([P, T2 * P], F32, name="h0buf", tag="h0buf")

        # h0 gathers (narrow indirect, validated primitive)
        for u in range(T2):
            nc.gpsimd.indirect_dma_start(
                out=h0buf[:, u * P : (u + 1) * P],
                out_offset=None,
                in_=emb_d[:],
                in_offset=AX(ap=h0m[:, u : u + 1], axis=0),
            )

        # PSUM: 6 mid banks + 1 seed bank + 1 wout bank
        NB = 6
        agg2_ps = ppool.tile([P, 4 * P], F32, tag="agg2", bufs=1, name="agg2ps")

        def psum_slice(pass_tiles, r):
            if r >= T1:  # seed rtile
                u = r - T1
                return agg2_ps[:, u * P : (u + 1) * P]
            lt = r - pass_tiles[0]
            bk, q = lt // 4, lt % 4
            return pass_banks[bk][:, q * P : (q + 1) * P]

        def mid_epilogue(t):
            zp = ppool.tile([P, P], F32, tag="wout", bufs=1)
            nc.tensor.matmul(
                out=zp[:], lhsT=aggsb[:, t * P : (t + 1) * P], rhs=W0_s[:],
                start=True, stop=True,
            )
            hs = spool.tile([P, P], BF16, tag="hs")
            if debug_dump:
                hsf = spool.tile([P, P], F32, tag="hsf")
            if has_b0:
                zt = spool.tile([P, P], F32, tag="zt")
                nc.scalar.activation(out=zt[:], in_=zp[:], func=CP,
                                     scale=r1m[:, t : t + 1])
                nc.vector.tensor_tensor(out=zt[:], in0=zt[:], in1=b0bc[:],
                                        op=mybir.AluOpType.add)
                nc.scalar.activation(out=hs[:], in_=zt[:], func=LR, alpha=ALPHA)
            else:
                nc.scalar.activation(out=hs[:], in_=zp[:], func=LR, alpha=ALPHA,
                                     scale=r1m[:, t : t + 1])
            nc.sync.dma_start(out=hsum_d[t * P : (t + 1) * P, :], in_=hs[:])
            if debug_dump:
                nc.vector.tensor_copy(out=hsf[:], in_=hs[:])
                nc.sync.dma_start(out=hsum_dbg_d[t * P : (t + 1) * P, :], in_=hsf[:])

        for pss in range(npasses):
            if pss == 0:
                pass_tiles = list(range(0, n_pass0))
            else:
                pass_tiles = list(range(n_pass0, T1))
            nb_used = (len(pass_tiles) + 3) // 4
            pass_banks = [
                ppool.tile([P, 4 * P], F32, tag=f"aggb{i}", bufs=1,
                           name=f"aggb{i}_p{pss}")
                for i in range(nb_used)
            ]
            for ci, call in enumerate(plan["calls"]):
                if call["pss"] != pss:
                    continue
                c0, cols, b = call["c0"], call["cols"], call["bank"]
                xg = wpool.tile([P, CMAX, P], F32, tag="xg")
                hi = min((b + 1) * BANK, emb_rows)
                nc.gpsimd.dma_gather(
                    out_ap=xg[:, :cols, :],
                    in_ap=emb_d[b * BANK : hi, :],
                    idxs_ap=idx_all[:, c0 * 8 : (c0 + cols) * 8],
                    num_idxs=cols * P,
                    num_idxs_reg=cols * P,
                    elem_size=D,
                    single_packet=False,
                )
                y = wpool.tile([P, CMAX, P], BF16, tag="y")
                nc.scalar.activation(out=y[:, :cols, :], in_=xg[:, :cols, :],
                                     func=LR, alpha=ALPHA)
                nc.vector.tensor_tensor(
                    out=y[:, :cols, :], in0=y[:, :cols, :],
                    in1=s_all[:, c0 : c0 + cols].unsqueeze(2).to_broadcast(
                        [P, cols, P]),
                    op=mybir.AluOpType.mult,
                )
                Rt = []
                for k in range(call["nplanes"]):
                    lo, hi = call["k_rng"][k]
                    R = wpool.tile([P, CMAX, P], BF16, tag=f"R{k}", bufs=2)
                    nc.vector.tensor_tensor(
                        out=R[:, lo:hi, :],
                        in0=planes[k][:, c0 + lo : c0 + hi].unsqueeze(2).to_broadcast(
                            [P, hi - lo, P]),
                        in1=iota[:].unsqueeze(1).to_broadcast([P, hi - lo, P]),
                        op=mybir.AluOpType.is_equal,
                    )
                    Rt.append(R)
                if debug_dump and ci == 0:
                    nc.sync.dma_start(out=xg0_d[:, : cols * P], in_=xg[:, :cols, :])
                    y0f = cpool.tile([P, CMAX, P], F32, name="y0f", tag="y0f")
                    nc.vector.tensor_copy(out=y0f[:, :cols, :], in_=y[:, :cols, :])
                    nc.sync.dma_start(out=y0_d[:, : cols * P], in_=y0f[:, :cols, :])
                    nc.vector.tensor_copy(out=y0f[:, :cols, :], in_=Rt[0][:, :cols, :])
                    nc.sync.dma_start(out=R00_d[:, : cols * P], in_=y0f[:, :cols, :])
                for col in range(cols):
                    for k, r in call["col_rtiles"][col]:
                        st, sp = mm_flags[(ci, col, k)]
                        nc.tensor.matmul(
                            out=psum_slice(pass_tiles, r),
                            lhsT=y[:, col, :],
                            rhs=Rt[k][:, col, :],
                            start=st,
                            stop=sp,
                        )
            # pass flush + epilogues
            for i in range(nb_used):
                t0 = pass_tiles[0] + i * 4
                nt = min(4, pass_tiles[-1] + 1 - t0)
                nc.vector.tensor_copy(
                    out=aggsb[:, t0 * P : (t0 + nt) * P],
                    in_=pass_banks[i][:, : nt * P],
                )
            for t in pass_tiles:
                mid_epilogue(t)

        # ---- L2B ----
        xB = cpool.tile([P, n2sub, P], BF16, name="xB", tag="xB")
        nc.gpsimd.dma_gather(
            out_ap=xB[:], in_ap=hsum_d[:], idxs_ap=idxB[:],
            num_idxs=n2sub * P, num_idxs_reg=n2sub * P, elem_size=D,
            single_packet=False,
        )
        GW = 16
        for g0 in range(0, n2sub, GW):
            gw = min(GW, n2sub - g0)
            dgB = wpool.tile([P, GW, P], BF16, tag="dgB")
            nc.vector.tensor_tensor(
                out=dgB[:, :gw, :],
                in0=ident[:].unsqueeze(1).to_broadcast([P, gw, P]),
                in1=s2[:, g0 : g0 + gw].unsqueeze(2).to_broadcast([P, gw, P]),
                op=mybir.AluOpType.mult,
            )
            for j in range(gw):
                su = g0 + j
                u = int(tile_of_su2[su])
                k = su - off2[u]
                nc.tensor.matmul(
                    out=agg2_ps[:, u * P : (u + 1) * P],
                    lhsT=xB[:, su, :],
                    rhs=dgB[:, j, :],
                    start=False,
                    stop=(su == n2sub - 1),
                )
        nc.vector.tensor_copy(out=agg2sb[:], in_=agg2_ps[:, : T2 * P])
        if debug_dump:
            aggf = cpool.tile([P, T1 * P], F32, name="aggf", tag="aggf")
            nc.vector.tensor_copy(out=aggf[:], in_=aggsb[:])
            nc.sync.dma_start(out=aggsb_d[:], in_=aggf[:])
            xBf = cpool.tile([P, n2sub, P], F32, name="xBf", tag="xBf")
            nc.vector.tensor_copy(out=xBf[:], in_=xB[:])
            nc.sync.dma_start(out=xB_dbg_d[:], in_=xBf[:])
            ag2f = cpool.tile([P, T2 * P], F32, name="ag2f", tag="ag2f")
            nc.vector.tensor_copy(out=ag2f[:], in_=agg2sb[:])
            nc.sync.dma_start(out=agg2_dbg_d[:], in_=ag2f[:])
        for u in range(T2):
            op_ = ppool.tile([P, P], F32, tag="wout", bufs=1)
            nc.tensor.matmul(
                out=op_[:], lhsT=agg2sb[:, u * P : (u + 1) * P], rhs=W1_s[:],
                start=True, stop=True,
            )
            ot = spool.tile([P, P], F32, tag="ot")
            nc.scalar.activation(out=ot[:], in_=op_[:], func=CP,
                                 scale=r2m[:, u : u + 1])
            nc.vector.tensor_tensor(
                out=ot[:], in0=ot[:], in1=h0buf[:, u * P : (u + 1) * P],
                op=mybir.AluOpType.add,
            )
            if has_b1:
                nc.vector.tensor_tensor(out=ot[:], in0=ot[:], in1=b1bc[:],
                                        op=mybir.AluOpType.add)
            nc.sync.dma_start(out=out_d[u * P : (u + 1) * P, :], in_=ot[:])

    nc.compile()
    return nc


def make_in_maps(inputs, plan, has_b0, has_b1):
    emb = np.ascontiguousarray(np.asarray(inputs["emb"], np.float32))
    W0bf = np.asarray(inputs["W0"], np.float32).astype(NP_BF16)
    W1bf = np.asarray(inputs["W1"], np.float32).astype(NP_BF16)
    identbf = np.eye(P, dtype=NP_BF16)
    iotabf = np.broadcast_to(np.arange(P, dtype=NP_BF16), (P, P)).copy()
    in_maps = []
    for pc in plan["cores"]:
        m = dict(
            emb=emb, W0bf=W0bf, W1bf=W1bf, identbf=identbf, iotabf=iotabf,
            idx_all=pc["idx_all"],
            s_all=pc["s_all"].astype(NP_BF16),
            idxB=pc["idxB"],
            s2bf=pc["s2meta"].astype(NP_BF16),
            r1meta=pc["r1meta"],
            h0meta=pc["h0meta"],
            r2meta=pc["r2meta"],
        )
        for k in range(plan["NPL"]):
            m[f"plane{k}"] = pc["planes"][k].astype(NP_BF16)
        if has_b0:
            m["b0bc"] = np.broadcast_to(
                np.asarray(inputs["b0"], np.float32), (P, D)).copy()
        if has_b1:
            m["b1bc"] = np.broadcast_to(
                2.0 * np.asarray(inputs["b1"], np.float32), (P, D)).copy()
        in_maps.append(m)
    return in_maps


def assemble_output(plan, core_outs):
    out = np.zeros((plan["B"], D), np.float32)
    for c, pc in enumerate(plan["cores"]):
        co = core_outs[c]
        for u in range(plan["T2"]):
            out[pc["orig_seeds"][u]] = co[u * P : (u + 1) * P]
    return out


_CACHE = {}


def _plan_key(inputs):
    h = hashlib.sha1()
    for k in ("nid_src1", "nid_src2", "nid_dst2", "e1_src", "e1_dst", "e2_src",
              "e2_dst", "b0", "b1"):
        a = np.ascontiguousarray(np.asarray(inputs[k]))
        h.update(k.encode())
        h.update(str(a.shape).encode())
        h.update(a.tobytes())
    return h.hexdigest()


def _get_compiled(inputs):
    key = _plan_key(inputs)
    if key not in _CACHE:
        pl = build_plan(inputs)
        has_b0 = bool(np.any(np.asarray(inputs["b0"]) != 0))
        has_b1 = bool(np.any(np.asarray(inputs["b1"]) != 0))
        nc = build_nc(pl, has_b0, has_b1)
        _CACHE[key] = (pl, has_b0, has_b1, nc)
    return _CACHE[key]


def run_kernel(inputs, trace=False, tmpdir=None):
    pl, has_b0, has_b1, nc = _get_compiled(inputs)
    in_maps = make_in_maps(inputs, pl, has_b0, has_b1)
    res = run_bass_kernel_spmd(
        nc, in_maps, core_ids=list(range(pl["ncores"])), trace=trace, tmpdir=tmpdir
    )
    core_outs = [res.results[c]["out"] for c in range(pl["ncores"])]
    out = assemble_output(pl, core_outs)
    return out, res


def kernel(**inputs):
    out, _ = run_kernel(inputs, trace=False)
    return out
